# revision 11
# baseline (speedup 1.0000x reference)
"""Trainium2 Bass kernel for nn_NeighbourAssignment.

Math (per edge e with target node n=idx[e]):
    logits = own @ Wt + src @ Ws + (bt + bs)          (E, 4)
    a      = softmax(logits)                          (E, 4)
    out[s, n, :] = (sum_e a[e,s] * (src[e] @ W_bank[s] + b_bank[s])) / max(cnt[n], 1)

Key algebraic restructure: the segment-sum commutes with the per-edge GEMM:
    G_s[n, :]  = sum_{e->n} a[e,s] * src[e, :]        (N, CS)   <- scatter, on PE
    m_s[n]     = sum_{e->n} a[e,s]                    (N,)
    out[s,n,:] = (G_s[n] @ W_bank[s] + m_s[n] * b_bank[s]) / cnt
This cuts FLOPs ~16x vs materializing (SIZE, E, OUT).

Sharding: edges sorted by node; nodes split evenly across 8 cores
(node-contiguous, so each core owns a disjoint node range and there is no
cross-core reduction; host just concatenates node slices).

Scatter on PE: edges packed into 32-node windows; per 128-edge tile a
(128e x 128) stationary matrix A_cat = [a_0*onehot | a_1*onehot | a_2*onehot |
a_3*onehot] (onehot: edge -> node-within-window) multiplies the (128e, 256c)
src tile, accumulating G for all 4 banks in one matmul stream.
"""
import os
from contextlib import ExitStack

import numpy as np

P = 128
WIN = 32  # nodes per scatter window
SIZE = 4


def _pack_windows(deg, W, cap_edges):
    """First-fit-decreasing bin-pack of nodes (weights=deg) into W windows of
    <= WIN nodes and <= cap_edges edges. Returns slot_of_node or None."""
    n = deg.shape[0]
    order = np.argsort(-deg, kind="stable")
    edges_used = np.zeros(W, dtype=np.int64)
    nodes_used = np.zeros(W, dtype=np.int64)
    slot = np.full(n, -1, dtype=np.int64)
    for nd in order:
        d = int(deg[nd])
        placed = False
        for b in range(W):
            if nodes_used[b] < WIN and edges_used[b] + d <= cap_edges:
                slot[nd] = b * WIN + nodes_used[b]
                nodes_used[b] += 1
                edges_used[b] += d
                placed = True
                break
        if not placed:
            return None
    return slot


def _build_plan(idx, n_nodes, n_cores):
    """Shard nodes across cores; within each core, bin-pack nodes into
    32-node windows so every window needs at most T 128-edge tiles."""
    nodes_per_core = -(-n_nodes // n_cores)  # ceil

    order = np.argsort(idx, kind="stable")
    sidx = idx[order]
    core_of = np.minimum(sidx // nodes_per_core, n_cores - 1)
    core_starts = np.searchsorted(core_of, np.arange(n_cores + 1))
    local = sidx - core_of * nodes_per_core

    # candidate (W, T) configs, smallest E_pad = W*T*128 first
    base_W = -(-nodes_per_core // WIN)
    base_W = -(-base_W // SIZE) * SIZE  # keep n_pad a multiple of 128
    cands = []
    for W_try in range(base_W, base_W + 4 * SIZE + 1, SIZE):
        for T_try in range(2, 8):
            if W_try * WIN >= nodes_per_core:
                cands.append((W_try * T_try * P, W_try, T_try))
    cands.sort()

    max_load = max(
        int(core_starts[c + 1] - core_starts[c]) for c in range(n_cores)
    )
    for E_pad, W, T in cands:
        if W * T * P < max_load:
            continue
        slots = []
        ok = True
        for c in range(n_cores):
            lo, hi = core_starts[c], core_starts[c + 1]
            deg = np.bincount(local[lo:hi], minlength=nodes_per_core)
            s = _pack_windows(deg, W, T * P)
            if s is None:
                ok = False
                break
            slots.append(s)
        if ok:
            break
    else:
        raise RuntimeError("window packing failed")

    n_pad = W * WIN
    plans = []
    for c in range(n_cores):
        lo, hi = core_starts[c], core_starts[c + 1]
        slot_of = slots[c]
        eslot = slot_of[local[lo:hi]]  # window slot of each edge
        win = eslot // WIN
        # order core's edges by slot; place into per-window tile ranges
        eorder = np.argsort(eslot, kind="stable")
        cnts = np.bincount(win[eorder], minlength=W)
        dst = np.empty(hi - lo, dtype=np.int64)
        off = 0
        for w in range(W):
            cw = int(cnts[w])
            dst[eorder[off : off + cw]] = w * T * P + np.arange(cw)
            off += cw
        perm = order[lo:hi]  # global edge id for each core-local edge
        o_reb = (eslot - win * WIN).astype(np.float32)
        plans.append({"perm": perm, "dst": dst, "o_val": o_reb,
                      "slot_of": slot_of})
    return {
        "plans": plans,
        "nodes_per_core": nodes_per_core,
        "n_pad": n_pad,
        "W": W,
        "T": T,
        "E_pad": E_pad,
        "core_starts": core_starts,
    }


def _host_prep(own, src, Wt, bt, Ws, bs, W_bank, b_bank, idx, n_nodes, n_cores):
    E = idx.shape[0]
    plan = _build_plan(idx, n_nodes, n_cores)
    npc, n_pad, W, T, E_pad = (
        plan["nodes_per_core"], plan["n_pad"], plan["W"], plan["T"], plan["E_pad"],
    )
    n_tiles = E_pad // P
    counts = np.bincount(idx, minlength=n_nodes).astype(np.float32)
    inv_full = 1.0 / np.maximum(counts, 1.0)

    # logits weight pack: [Wt_c0 | Wt_c1 | Ws_c0 | Ws_c1], each (128, 4)
    Wlog = np.concatenate(
        [Wt[0:P], Wt[P : 2 * P], Ws[0:P], Ws[P : 2 * P]], axis=1
    ).astype(np.float32)  # (128, 16)
    ebb = np.tile(np.exp((bt + bs).astype(np.float64)).astype(np.float32), T)
    ebb = np.tile(ebb[None, :], (P, 1))  # (128, 4*T)
    # final GEMM weights: per (s, chunk): W_bank[s, ch*128:(ch+1)*128, :] (128, 256)
    Wb = np.concatenate(
        [W_bank[s, ch * P : (ch + 1) * P, :] for s in range(SIZE) for ch in range(2)],
        axis=1,
    ).astype(np.float32)  # (128, 2048)
    bb = np.tile(b_bank.reshape(1, SIZE * 256), (P, 1)).astype(np.float32)  # (128,1024)

    in_maps = []
    for c in range(n_cores):
        p = plan["plans"][c]
        perm, dst, o_val, slot_of = p["perm"], p["dst"], p["o_val"], p["slot_of"]

        src_pad = np.zeros((E_pad, 256), dtype=np.float32)
        src_pad[dst] = src[perm]
        own_pad = np.zeros((E_pad, 256), dtype=np.float32)
        own_pad[dst] = own[perm]
        o_reb = np.full(E_pad, -1.0, dtype=np.float32)
        o_reb[dst] = o_val

        # per-slot 1/count (pad slots -> 1)
        nreal = min(npc, n_nodes - c * npc)
        inv_slot = np.ones(n_pad, dtype=np.float32)
        inv_slot[slot_of[:nreal]] = inv_full[c * npc : c * npc + nreal]
        # invW: (128 = 4s x 32j, W): inv for slot 32w + j  (same for each s)
        j = np.arange(P) % WIN
        invW = inv_slot[(np.arange(W)[None, :] * WIN) + j[:, None]].astype(np.float32)
        # invG: (128, n_pad//128): inv for slot 128g + p
        invG = inv_slot.reshape(n_pad // P, P).T.copy()

        in_maps.append({
            "srcD": src_pad,
            "ownTD": np.ascontiguousarray(own_pad.T),
            "srcTD": np.ascontiguousarray(src_pad.T),
            "oD": np.ascontiguousarray(o_reb.reshape(n_tiles, P).T),
            "WlogD": Wlog,
            "ebbD": ebb,
            "WbD": Wb,
            "bbD": bb,
            "invWD": invW,
            "invGD": invG,
        })
    return plan, in_maps


def _build_program(E_pad, n_pad, W, T):
    import concourse.bacc as bacc
    import concourse.tile as tile
    from concourse import mybir
    from concourse.masks import make_identity

    dt = mybir.dt
    f32 = dt.float32
    AF = mybir.ActivationFunctionType
    OP = mybir.AluOpType

    n_tiles = E_pad // P
    G = n_pad // P  # node groups of 128

    nc = bacc.Bacc("TRN2", target_bir_lowering=False, debug=False)

    srcD = nc.dram_tensor("srcD", [E_pad, 256], f32, kind="ExternalInput").ap()
    ownTD = nc.dram_tensor("ownTD", [256, E_pad], f32, kind="ExternalInput").ap()
    srcTD = nc.dram_tensor("srcTD", [256, E_pad], f32, kind="ExternalInput").ap()
    oD = nc.dram_tensor("oD", [P, n_tiles], f32, kind="ExternalInput").ap()
    WlogD = nc.dram_tensor("WlogD", [P, 16], f32, kind="ExternalInput").ap()
    ebbD = nc.dram_tensor("ebbD", [P, 4 * T], f32, kind="ExternalInput").ap()
    WbD = nc.dram_tensor("WbD", [P, 2048], f32, kind="ExternalInput").ap()
    bbD = nc.dram_tensor("bbD", [P, 1024], f32, kind="ExternalInput").ap()
    invWD = nc.dram_tensor("invWD", [P, W], f32, kind="ExternalInput").ap()
    invGD = nc.dram_tensor("invGD", [P, G], f32, kind="ExternalInput").ap()
    outD = nc.dram_tensor("outD", [SIZE, n_pad, 256], f32, kind="ExternalOutput").ap()

    with tile.TileContext(nc) as tc, ExitStack() as ctx:
        cst = ctx.enter_context(tc.tile_pool(name="cst", bufs=1))
        lg_in = ctx.enter_context(tc.tile_pool(name="lg_in", bufs=3))
        sm = ctx.enter_context(tc.tile_pool(name="sm", bufs=4))
        srcp = ctx.enter_context(tc.tile_pool(name="srcp", bufs=6))
        ap_ = ctx.enter_context(tc.tile_pool(name="ap", bufs=6))
        gsb = ctx.enter_context(tc.tile_pool(name="gsb", bufs=3))
        outp = ctx.enter_context(tc.tile_pool(name="outp", bufs=3))
        # psum pools (8 banks total: 2+2+2+2)
        ps_a = ctx.enter_context(tc.tile_pool(name="ps_a", bufs=2, space="PSUM"))
        ps_g = ctx.enter_context(tc.tile_pool(name="ps_g", bufs=2, space="PSUM"))
        ps_m = ctx.enter_context(tc.tile_pool(name="ps_m", bufs=2, space="PSUM"))
        ps_t = ctx.enter_context(tc.tile_pool(name="ps_t", bufs=2, space="PSUM"))

        # ---- constants ----
        iota32 = cst.tile([P, WIN], f32, tag="iota32")
        nc.gpsimd.iota(iota32[:], pattern=[[1, WIN]], base=0, channel_multiplier=0,
                       allow_small_or_imprecise_dtypes=True)
        ident = cst.tile([P, P], f32, tag="ident")
        make_identity(nc, ident[:])
        Wlog = cst.tile([P, 16], f32, tag="Wlog")
        nc.sync.dma_start(Wlog[:], WlogD[:])
        ebb = cst.tile([P, 4 * T], f32, tag="ebb")
        nc.sync.dma_start(ebb[:], ebbD[:])
        Wb = cst.tile([P, 2048], f32, tag="Wb")
        nc.sync.dma_start(Wb[:], WbD[:])
        bb = cst.tile([P, 1024], f32, tag="bb")
        nc.sync.dma_start(bb[:], bbD[:])
        invW = cst.tile([P, W], f32, tag="invW")
        nc.sync.dma_start(invW[:], invWD[:])
        invG = cst.tile([P, G], f32, tag="invG")
        nc.sync.dma_start(invG[:], invGD[:])
        o_all = cst.tile([P, n_tiles], f32, tag="o_all")
        nc.sync.dma_start(o_all[:], oD[:])
        a_all = cst.tile([P, 4 * n_tiles], f32, tag="a_all")
        # persistent outputs of stage B->C
        gtA = cst.tile([P, SIZE * n_pad], f32, tag="gtA")  # G^T chunk0 (c 0:128)
        gtB = cst.tile([P, SIZE * n_pad], f32, tag="gtB")  # G^T chunk1 (c 128:256)
        mT = cst.tile([4, n_pad], f32, tag="mT")

        gtA_v = gtA[:].rearrange("p (s n) -> p s n", s=SIZE)
        gtB_v = gtB[:].rearrange("p (s n) -> p s n", s=SIZE)

        # ---- stages A+B interleaved per window (1-window software pipeline):
        # logits/softmax of window w+1 are emitted before the scatter of
        # window w, so scatter matmuls hide the logits weight-loads and the
        # PE never stalls on the softmax ACT/DVE chain.
        EW = T * P  # edges per window

        ownT_view = ownTD[:].rearrange("(u p) e -> p u e", p=P)
        srcT_view = srcTD[:].rearrange("(u p) e -> p u e", p=P)

        def emit_logits(w):
            e0 = w * EW
            ownT = lg_in.tile([P, 2 * EW], f32, tag="ownT")
            srcT = lg_in.tile([P, 2 * EW], f32, tag="srcT")
            ownT_v = ownT[:].rearrange("p (u e) -> p u e", u=2)
            srcT_v = srcT[:].rearrange("p (u e) -> p u e", u=2)
            nc.sync.dma_start(ownT_v, ownT_view[:, :, e0 : e0 + EW])
            nc.sync.dma_start(srcT_v, srcT_view[:, :, e0 : e0 + EW])

            lg = ps_a.tile([P, 4 * T], f32, tag="lg")
            for ti in range(T):
                sl = slice(128 * ti, 128 * ti + 128)
                co = slice(4 * ti, 4 * ti + 4)
                nc.tensor.matmul(lg[:, co], lhsT=ownT[:, sl], rhs=Wlog[:, 0:4],
                                 start=True, stop=False)
                nc.tensor.matmul(lg[:, co], lhsT=ownT[:, EW + 128 * ti : EW + 128 * ti + 128],
                                 rhs=Wlog[:, 4:8], start=False, stop=False)
                nc.tensor.matmul(lg[:, co], lhsT=srcT[:, sl], rhs=Wlog[:, 8:12],
                                 start=False, stop=False)
                nc.tensor.matmul(lg[:, co], lhsT=srcT[:, EW + 128 * ti : EW + 128 * ti + 128],
                                 rhs=Wlog[:, 12:16], start=False, stop=True)

            expt = sm.tile([P, 4 * T], f32, tag="expt")
            nc.scalar.activation(expt[:], lg[:], AF.Exp)
            # fold the (bt+bs) bias in as exp(logit)*exp(b) (ebb host const)
            nc.vector.tensor_tensor(out=expt[:], in0=expt[:], in1=ebb[:],
                                    op=OP.mult)
            Z = sm.tile([P, T], f32, tag="Z")
            nc.vector.tensor_reduce(
                Z[:], expt[:].rearrange("p (t s) -> p t s", s=4),
                axis=mybir.AxisListType.X, op=OP.add,
            )
            rZ = sm.tile([P, T], f32, tag="rZ")
            nc.vector.reciprocal(rZ[:], Z[:])
            for ti in range(T):
                g = w * T + ti
                nc.vector.tensor_scalar(
                    out=a_all[:, 4 * g : 4 * g + 4],
                    in0=expt[:, 4 * ti : 4 * ti + 4],
                    scalar1=rZ[:, ti : ti + 1], scalar2=None, op0=OP.mult,
                )

        srcD_view = srcD[:].rearrange("(g p) c -> p g c", p=P)

        def emit_scatter(w):
            g_ps = ps_g.tile([P, 256], f32, tag="g_ps")
            mT_ps = ps_m.tile([4, WIN], f32, tag="mT_ps")
            src_w = srcp.tile([P, T * 256], f32, tag="src_w")
            src_w_v = src_w[:].rearrange("p (t c) -> p t c", t=T)
            nc.sync.dma_start(src_w_v, srcD_view[:, w * T : w * T + T, :])
            for t in range(T):
                g = w * T + t
                src_t = src_w_v[:, t, :]
                oh = ap_.tile([P, WIN], f32, tag="oh")
                nc.gpsimd.tensor_scalar(
                    out=oh[:], in0=iota32[:], scalar1=o_all[:, g : g + 1],
                    scalar2=None, op0=OP.is_equal,
                )
                A_cat = ap_.tile([P, P], f32, tag="A_cat")
                nc.vector.tensor_tensor(
                    out=A_cat[:].rearrange("p (s j) -> p s j", s=SIZE),
                    in0=oh[:].unsqueeze(1).broadcast_to([P, SIZE, WIN]),
                    in1=a_all[:, 4 * g : 4 * g + 4].unsqueeze(2)
                        .broadcast_to([P, SIZE, WIN]),
                    op=OP.mult,
                )
                nc.tensor.matmul(g_ps[:], lhsT=A_cat[:], rhs=src_t,
                                 start=(t == 0), stop=(t == T - 1))
                nc.tensor.matmul(mT_ps[:], lhsT=a_all[:, 4 * g : 4 * g + 4],
                                 rhs=oh[:], start=(t == 0), stop=(t == T - 1))
            # evacuate: G scaled by 1/count; mT plain
            g_sb = gsb.tile([P, 256], f32, tag="g_sb")
            nc.scalar.activation(g_sb[:], g_ps[:], AF.Copy, bias=0.0,
                                 scale=invW[:, w : w + 1])
            nc.scalar.copy(mT[0:4, WIN * w : WIN * w + WIN], mT_ps[:])
            # transpose the two 128-col chunks; de-interleave into gtA/gtB
            for ch, gt_v in ((0, gtA_v), (1, gtB_v)):
                tp = ps_t.tile([P, P], f32, tag="tp")
                nc.tensor.transpose(tp[:], g_sb[:, 128 * ch : 128 * ch + 128],
                                    ident[:])
                nc.scalar.copy(
                    gt_v[:, :, WIN * w : WIN * w + WIN],
                    tp[:].rearrange("p (s j) -> p s j", s=SIZE),
                )

        emit_logits(0)
        for w in range(W):
            if w + 1 < W:
                emit_logits(w + 1)
            emit_scatter(w)

        # ---- stage C: final GEMM + bias + writeback ----
        for g in range(G):
            mnp = ps_m.tile([P, 4], f32, tag="mT_ps")  # share slots with mT_ps
            nc.tensor.transpose(mnp[:], mT[0:4, P * g : P * g + P], ident[0:4, 0:4])
            m_sb = sm.tile([P, 4], f32, tag="m_sb")
            nc.vector.tensor_scalar(out=m_sb[:], in0=mnp[:],
                                    scalar1=invG[:, g : g + 1], scalar2=None,
                                    op0=OP.mult)
            for s in range(SIZE):
                o_ps = ps_a.tile([P, 256], f32, tag="lg")  # share slots with lg
                nc.tensor.matmul(o_ps[:], lhsT=gtA_v[:, s, P * g : P * g + P],
                                 rhs=Wb[:, (2 * s) * 256 : (2 * s) * 256 + 256],
                                 start=True, stop=False)
                nc.tensor.matmul(o_ps[:], lhsT=gtB_v[:, s, P * g : P * g + P],
                                 rhs=Wb[:, (2 * s + 1) * 256 : (2 * s + 1) * 256 + 256],
                                 start=False, stop=True)
                o_sb = outp.tile([P, 256], f32, tag="o_sb")
                nc.vector.scalar_tensor_tensor(
                    out=o_sb[:], in0=bb[:, 256 * s : 256 * s + 256],
                    scalar=m_sb[:, s : s + 1], in1=o_ps[:],
                    op0=OP.mult, op1=OP.add,
                )
                nc.sync.dma_start(outD[s, P * g : P * g + P, :], o_sb[:])

    nc.compile()
    return nc


_PROG_CACHE = {}


def kernel(own_data, source_message, Wt, bt, Ws_assign, bs_assign,
           W_bank, b_bank, indices, node_count, _trace=False):
    from concourse.bass_utils import run_bass_kernel_spmd

    own = np.asarray(own_data, dtype=np.float32)
    src = np.asarray(source_message, dtype=np.float32)
    Wt = np.asarray(Wt, dtype=np.float32)
    bt = np.asarray(bt, dtype=np.float32)
    Ws = np.asarray(Ws_assign, dtype=np.float32)
    bs = np.asarray(bs_assign, dtype=np.float32)
    W_bank = np.asarray(W_bank, dtype=np.float32)
    b_bank = np.asarray(b_bank, dtype=np.float32)
    idx = np.asarray(indices).astype(np.int64)
    N = int(node_count)
    n_cores = 8

    plan, in_maps = _host_prep(own, src, Wt, bt, Ws, bs, W_bank, b_bank,
                               idx, N, n_cores)
    key = (plan["E_pad"], plan["n_pad"], plan["W"], plan["T"])
    if key not in _PROG_CACHE:
        _PROG_CACHE[key] = _build_program(*key)
    nc = _PROG_CACHE[key]

    res = run_bass_kernel_spmd(nc, in_maps, core_ids=list(range(n_cores)),
                               trace=_trace)
    npc = plan["nodes_per_core"]
    out = np.empty((SIZE, N, 256), dtype=np.float32)
    for c in range(n_cores):
        nreal = min(npc, N - c * npc)
        slot_of = plan["plans"][c]["slot_of"]
        out[:, c * npc : c * npc + nreal, :] = (
            res.results[c]["outD"][:, slot_of[:nreal], :]
        )
    if _trace and res.exec_time_ns is not None:
        print(f"HW exec time: {res.exec_time_ns} ns")
    kernel._last_result = res
    return out
